# revision 1
# baseline (speedup 1.0000x reference)
"""Trainium2 Bass kernel for nn_EnhancedReflectiveCognitiveGraph (GNN edge-softmax attention).

Math (see reference):
  q/k/v = x @ W{q,k,v}.T + b ; per-edge scores s_e = <q[src_e], k[dest_e]>_head / 4
  softmax over edges sharing src (max-subtraction skipped: scores ~ N(0,1) so
  exp never overflows in fp32/fp16 and the weights are mathematically identical)
  agg[dest] += w_e * v[src_e] ; out = agg @ Wo.T + bo

Device strategy (8 cores, node-range sharding, three SPMD NEFF launches):
  L1 (proj):  each core computes q/k/v (fp16) for its node shard.  Host
      assembles the full k table (relayout only).
  L2 (src phase): core c owns edges with src in its shard, laid out in
      128-edge chunks, uniform across cores: chunk -> (dest-half, src-block)
      map identical on every core so one program serves all 8.  q rows are
      expanded per-edge ON-CHIP via PE matmuls against host-streamed one-hot
      matrices (S); k rows fetched with dma_gather (int16 indices, so the k
      table is addressed as lo/hi halves); scores -> exp -> per-src-block
      segment sums via PE matmuls with streamed S^T; recip -> u = recip * v
      ("u-table" trick: folds the softmax denominator into the value rows so
      the dest phase needs no per-edge denominator gather).
  L3 (dest phase): core c owns edges with dest in its shard.  u rows fetched
      with dma_gather, weighted by (host-permuted) exp-scores, scatter-added
      into per-dest-block agg via PE matmuls with streamed one-hots (T^T),
      then the output projection.  agg is complete locally (dest-sharded):
      no collectives and no racy HBM scatter-adds anywhere.
  Host between launches does pure relayout (concat / permute / pad / zero).
"""

import math
import ml_dtypes
import numpy as np

import concourse.bacc as bacc
import concourse.mybir as mybir
import concourse.tile as tile
from concourse.bass_utils import run_bass_kernel_spmd

# ---------------------------------------------------------------- constants
N = 50000
E = 600000
F = 128
H = 8
Dh = 16
P = 128
C = 8                     # cores
SH = 6272                 # nodes per core, cores 0-6 (49 blocks); core 7: 6096
NB = 49                   # blocks per shard (common; core 7 block 48 is empty)
LOHI = 32768              # int16 index split point
NPAD = 50176              # padded gather-table rows (multiple of 128)
GB = 64                   # chunks per gather batch (needs single_packet=False:
                          # single-packet dma_gather caps at ~1024 descs on HW)
SB = 64                   # chunks per one-hot stream DMA batch
PB = 12                   # chunks per PSUM/DVE batch (qe 3 banks x2 + seg 2 = 8)
F16 = mybir.dt.float16
F8 = mybir.dt.float8e4
F32 = mybir.dt.float32
I16 = mybir.dt.int16


def shard_base(c):
    return c * SH


def shard_len(c):
    return min(N, (c + 1) * SH) - c * SH


# ---------------------------------------------------------------- host prep
def pack_idx16(idx):
    """int16 dma_gather index layout: slot i -> partition i%16, col i//16,
    replicated across the 8 groups of 16 partitions."""
    n = len(idx)
    cols = (n + 15) // 16
    flat = np.zeros(16 * cols, dtype=np.int16)
    flat[:n] = idx
    arr = flat.reshape(cols, 16).T.copy()
    return np.tile(arr, (8, 1))


class ChunkMap:
    """Uniform chunk structure shared by all cores for one phase.

    Chunks (128 slots each) are laid out [all lo-half | all hi-half]; within a
    half, K[half] chunks per block, block-major.  chunk -> (half, block) is
    data-independent; only slot contents differ per core."""

    def __init__(self, k_lo, k_hi):
        self.k = (k_lo, k_hi)
        self.chunks = [(hf, b) for hf in (0, 1) for b in range(NB)
                       for _ in range(self.k[hf])]
        self.nch = len(self.chunks)
        self.nslots = self.nch * P
        self.n_lo_chunks = NB * k_lo

    def region_len(self, c0):
        """chunks remaining in c0's (lo/hi) region starting at c0."""
        end = self.n_lo_chunks if c0 < self.n_lo_chunks else self.nch
        return end - c0

    def gather_calls(self):
        """(start_chunk, n_chunks, half): GB-chunk batches, region-aligned."""
        calls = []
        for lohi, a, b in ((0, 0, self.n_lo_chunks), (1, self.n_lo_chunks, self.nch)):
            c = a
            while c < b:
                n = min(GB, b - c)
                calls.append((c, n, lohi))
                c += n
        return calls


class CorePlan:
    """Per-core slot contents for one phase.  `key` = node defining the block
    (src for L2, dest for L3); `other` = node indexing the gather table."""

    def __init__(self, cmap, core, key, other, edge_ids):
        base = shard_base(core)
        self.slot_local = np.full(cmap.nslots, -1, np.int64)
        self.slot_gidx = np.zeros(cmap.nslots, np.int64)
        self.slot_edge = np.full(cmap.nslots, -1, np.int64)
        half = (other >= LOHI).astype(np.int64)
        block = (key - base) // P
        # chunk start slot for each (half, block)
        start = {}
        pos = 0
        for hf in (0, 1):
            for b in range(NB):
                start[(hf, b)] = pos * P
                pos += cmap.k[hf]
        for hf in (0, 1):
            for b in range(NB):
                m = (half == hf) & (block == b)
                cnt = int(m.sum())
                if cnt == 0:
                    continue
                assert cnt <= cmap.k[hf] * P
                s0 = start[(hf, b)]
                self.slot_local[s0:s0 + cnt] = key[m] - base - b * P
                self.slot_gidx[s0:s0 + cnt] = other[m] - (LOHI if hf else 0)
                self.slot_edge[s0:s0 + cnt] = edge_ids[m]
        self.cmap = cmap

    def onehot_stream(self, transposed):
        """[128, nch*128] fp16; chunk c at cols c*128:(c+1)*128.
        transposed=False: S   [key_local, e] ; True: S^T [e, key_local].
        Dummy slots are all-zero columns/rows."""
        cm = self.cmap
        out = np.zeros((P, cm.nch * P), dtype=ml_dtypes.float8_e4m3)
        loc = self.slot_local
        sl_all = np.arange(cm.nslots)
        valid = loc >= 0
        ch = sl_all // P
        row = sl_all % P
        if transposed:
            out[row[valid], ch[valid] * P + loc[valid]] = 1.0
        else:
            out[loc[valid], ch[valid] * P + row[valid]] = 1.0
        return out


def compute_cmap(key, other):
    """Global uniform chunk counts per (half, block) for one phase."""
    k_lo = k_hi = 1
    for c in range(C):
        base, ln = shard_base(c), shard_len(c)
        m = (key >= base) & (key < base + ln)
        kk, oo = key[m], other[m]
        hf = (oo >= LOHI).astype(np.int64)
        blk = (kk - base) // P
        for hfv in (0, 1):
            cnt = np.bincount(blk[hf == hfv], minlength=NB)
            need = int(np.ceil(cnt.max() / P)) if cnt.size else 1
            if hfv == 0:
                k_lo = max(k_lo, need)
            else:
                k_hi = max(k_hi, need)
    return ChunkMap(k_lo, k_hi)


# ---------------------------------------------------------------- L1: projections
def build_l1():
    nc = bacc.Bacc("TRN2", target_bir_lowering=False, num_devices=C)
    xT = nc.dram_tensor("xT", [P, NB * P], F16, kind="ExternalInput")
    wqkv = nc.dram_tensor("wqkv", [P, 3 * P], F16, kind="ExternalInput")
    bqkv = nc.dram_tensor("bqkv", [1, 3 * P], F16, kind="ExternalInput")
    ones = nc.dram_tensor("ones", [1, P], F16, kind="ExternalInput")
    outs = {o: nc.dram_tensor(o, [NB * P, P], F16, kind="ExternalOutput")
            for o in ("q_sh", "k_sh", "v_sh")}

    with tile.TileContext(nc) as tc:
        with tc.tile_pool(name="const", bufs=1) as cpool, \
             tc.tile_pool(name="psum", bufs=4, space="PSUM") as ppool:
            w_sb = cpool.tile([P, 3 * P], F16, tag="w")
            nc.sync.dma_start(w_sb[:], wqkv[:])
            b_sb = cpool.tile([1, 3 * P], F16, tag="b")
            nc.sync.dma_start(b_sb[:], bqkv[:])
            ones_sb = cpool.tile([1, P], F16, tag="ones")
            nc.sync.dma_start(ones_sb[:], ones[:])
            xt = cpool.tile([P, NB * P], F16, tag="xT")
            nc.sync.dma_start(xt[:], xT[:])
            osb = cpool.tile([P, NB * 3 * P], F16, tag="osb")
            for b in range(NB):
                ps = ppool.tile([P, 3 * P], F32, tag="proj")
                nc.tensor.matmul(ps[:], lhsT=xt[:, b * P:(b + 1) * P],
                                 rhs=w_sb[:], start=True, stop=False)
                nc.tensor.matmul(ps[:], lhsT=ones_sb[:], rhs=b_sb[:],
                                 start=False, stop=True)
                nc.vector.tensor_copy(osb[:, b * 3 * P:(b + 1) * 3 * P], ps[:])
            osb4 = osb[:].rearrange("p (b t f) -> p b t f", t=3, f=P)
            for i, o in enumerate(("q_sh", "k_sh", "v_sh")):
                nc.sync.dma_start(
                    outs[o][:].rearrange("(b p) f -> p b f", p=P),
                    osb4[:, :, i, :])
    nc.compile()
    return nc


# ---------------------------------------------------------------- L2: src phase
def build_l2(cmap):
    nch, nsl = cmap.nch, cmap.nslots
    nc = bacc.Bacc("TRN2", target_bir_lowering=False, num_devices=C,
                   num_swdge_queues=2)
    q_sh = nc.dram_tensor("q_sh", [NB * P, P], F16, kind="ExternalInput")
    v_sh = nc.dram_tensor("v_sh", [NB * P, P], F16, kind="ExternalInput")
    k_full = nc.dram_tensor("k_full", [NPAD, P], F16, kind="ExternalInput")
    S_st = nc.dram_tensor("S_st", [P, nch * P], F8, kind="ExternalInput")
    ST_st = nc.dram_tensor("ST_st", [P, nch * P], F8, kind="ExternalInput")
    kidx = nc.dram_tensor("kidx", [P, nsl // 16], I16, kind="ExternalInput")
    exp_out = nc.dram_tensor("exp_out", [P, nch * H], F16, kind="ExternalOutput")
    u_out = nc.dram_tensor("u_out", [NB * P, P], F16, kind="ExternalOutput")

    with tile.TileContext(nc) as tc:
        with tile_pools(tc) as (rpool, spool, wpool, qpsum, gpsum):
            q_sb = rpool.tile([P, NB * P], F16, tag="q_sb")
            nc.sync.dma_start(
                q_sb[:].rearrange("p (b f) -> p b f", f=P),
                q_sh[:].rearrange("(b p) f -> p b f", p=P))
            v_sb = rpool.tile([P, NB * P], F16, tag="v_sb")
            nc.sync.dma_start(
                v_sb[:].rearrange("p (b f) -> p b f", f=P),
                v_sh[:].rearrange("(b p) f -> p b f", p=P))
            kidx_sb = rpool.tile([P, nsl // 16], I16, tag="kidx")
            nc.sync.dma_start(kidx_sb[:], kidx[:])
            exp_sb = rpool.tile([P, nch * H], F16, tag="exp_sb")
            seg_lo = rpool.tile([P, NB * H], F32, tag="seg_lo")
            seg_hi = rpool.tile([P, NB * H], F32, tag="seg_hi")
            nc.vector.memset(seg_lo[:], 0)
            nc.vector.memset(seg_hi[:], 0)

            kg_tiles = {}
            for qi, (c0, nch_c, lohi) in enumerate(cmap.gather_calls()):
                kg = spool.tile([P, GB * P], F16, tag="k_g")
                src_ap = k_full[0:LOHI, :] if lohi == 0 else k_full[LOHI:NPAD, :]
                nc.gpsimd.dma_gather(
                    out_ap=kg[:, :nch_c * P].rearrange("p (s f) -> p s f", f=P),
                    in_ap=src_ap,
                    idxs_ap=kidx_sb[:, c0 * P // 16:(c0 + nch_c) * P // 16],
                    num_idxs=nch_c * P,
                    num_idxs_reg=nch_c * P,
                    elem_size=P,
                    single_packet=False,
                    queue_num=qi % 2,
                )
                kg_tiles[c0] = kg

            s_tiles = {}
            st_tiles = {}

            def stream_tile(tiles, dram, ci):
                b0 = ci // SB * SB
                if b0 not in tiles:
                    t = spool.tile([P, SB * P], F8, tag=dram.name, name=f"strm_{dram.name}_{b0}")
                    n = min(SB, nch - b0) * P
                    nc.sync.dma_start(t[:, :n], dram[:, b0 * P:b0 * P + n])
                    tiles[b0] = t
                return tiles[b0][:, (ci - b0) * P:(ci - b0 + 1) * P]

            for cb0 in range(0, nch, PB):
                cbn = min(PB, nch - cb0)
                qe = qpsum.tile([P, PB * P], F32, tag="qe")
                for ci in range(cb0, cb0 + cbn):
                    blk = cmap.chunks[ci][1]
                    nc.tensor.matmul(
                        qe[:, (ci - cb0) * P:(ci - cb0 + 1) * P],
                        lhsT=stream_tile(s_tiles, S_st, ci),
                        rhs=q_sb[:, blk * P:(blk + 1) * P],
                        start=True, stop=True)
                qk = wpool.tile([P, PB * P], F16, tag="qk")
                sc = wpool.tile([P, PB * H], F32, tag="sc")
                ci = cb0
                while ci < cb0 + cbn:
                    gkey = max(s for s in kg_tiles if s <= ci)
                    cj = min(cb0 + cbn,
                             gkey + min(GB, cmap.region_len(gkey)))
                    n = cj - ci
                    off = (ci - gkey) * P
                    nc.vector.scalar_tensor_tensor(
                        out=qk[:, (ci - cb0) * P:(ci - cb0 + n) * P],
                        in0=qe[:, (ci - cb0) * P:(ci - cb0 + n) * P],
                        scalar=1.0,
                        in1=kg_tiles[gkey][:, off:off + n * P],
                        op0=mybir.AluOpType.mult,
                        op1=mybir.AluOpType.mult)
                    nc.vector.tensor_reduce(
                        out=sc[:, (ci - cb0) * H:(ci - cb0 + n) * H],
                        in_=qk[:, (ci - cb0) * P:(ci - cb0 + n) * P]
                        .rearrange("p (c h d) -> p c h d", h=H, d=Dh),
                        axis=mybir.AxisListType.X,
                        op=mybir.AluOpType.add)
                    ci = cj
                nc.scalar.activation(
                    out=exp_sb[:, cb0 * H:(cb0 + cbn) * H],
                    in_=sc[:, :cbn * H],
                    func=mybir.ActivationFunctionType.Exp,
                    scale=1.0 / math.sqrt(Dh))
                # segment-sum matmuls, grouped by (half, block)
                ci = cb0
                while ci < cb0 + cbn:
                    hf, blk = cmap.chunks[ci]
                    cj = ci
                    while cj + 1 < cb0 + cbn and cmap.chunks[cj + 1] == (hf, blk):
                        cj += 1
                    seg_ps = gpsum.tile([P, H], F32, tag="seg")
                    for ck in range(ci, cj + 1):
                        nc.tensor.matmul(
                            seg_ps[:],
                            lhsT=stream_tile(st_tiles, ST_st, ck),
                            rhs=exp_sb[:, ck * H:(ck + 1) * H],
                            start=(ck == ci), stop=(ck == cj))
                    acc = seg_lo if hf == 0 else seg_hi
                    nc.vector.tensor_add(
                        out=acc[:, blk * H:(blk + 1) * H],
                        in0=acc[:, blk * H:(blk + 1) * H],
                        in1=seg_ps[:])
                    ci = cj + 1

            seg = wpool.tile([P, NB * H], F32, tag="seg_tot", bufs=1)
            nc.vector.tensor_add(out=seg[:], in0=seg_lo[:], in1=seg_hi[:])
            rec_raw = wpool.tile([P, NB * H], F32, tag="rec_raw", bufs=1)
            nc.vector.reciprocal(rec_raw[:], seg[:])
            # zero-degree nodes / padding have seg == 0 -> 1/0 = inf; mask the
            # reciprocal to 0 there so fp16 u stays finite (rows never used).
            rec = wpool.tile([P, NB * H], F32, tag="rec", bufs=1)
            nc.vector.scalar_tensor_tensor(
                out=rec[:], in0=seg[:], scalar=0.0, in1=rec_raw[:],
                op0=mybir.AluOpType.is_gt, op1=mybir.AluOpType.mult)
            rrep = wpool.tile([P, NB * P], F16, tag="rrep", bufs=1)
            nc.scalar.copy(
                rrep[:].rearrange("p (b h d) -> p b h d", h=H, d=Dh),
                rec[:].rearrange("p (b h) -> p b h", h=H)[:, :, :, None]
                .broadcast_to([P, NB, H, Dh]))
            u_sb = wpool.tile([P, NB * P], F16, tag="u_sb", bufs=1)
            nc.vector.tensor_mul(u_sb[:], v_sb[:], rrep[:])
            nc.sync.dma_start(
                u_out[:].rearrange("(b p) f -> p b f", p=P),
                u_sb[:].rearrange("p (b f) -> p b f", f=P))
            nc.sync.dma_start(exp_out[:], exp_sb[:])
    nc.compile()
    return nc


def tile_pools(tc):
    import contextlib

    @contextlib.contextmanager
    def pools():
        with tc.tile_pool(name="resident", bufs=1) as rpool, \
             tc.tile_pool(name="stream", bufs=2) as spool, \
             tc.tile_pool(name="work", bufs=3) as wpool, \
             tc.tile_pool(name="big_psum", bufs=2, space="PSUM") as qpsum, \
             tc.tile_pool(name="small_psum", bufs=2, space="PSUM") as gpsum:
            yield rpool, spool, wpool, qpsum, gpsum
    return pools()


# ---------------------------------------------------------------- L3: dest phase
def build_l3(cmap):
    nch, nsl = cmap.nch, cmap.nslots
    nc = bacc.Bacc("TRN2", target_bir_lowering=False, num_devices=C,
                   num_swdge_queues=2)
    u_full = nc.dram_tensor("u_full", [NPAD, P], F16, kind="ExternalInput")
    TT_st = nc.dram_tensor("TT_st", [P, nch * P], F8, kind="ExternalInput")
    uidx = nc.dram_tensor("uidx", [P, nsl // 16], I16, kind="ExternalInput")
    exp_in = nc.dram_tensor("exp_in", [P, nch * H], F16, kind="ExternalInput")
    WoT = nc.dram_tensor("WoT", [P, P], F16, kind="ExternalInput")
    bo_r = nc.dram_tensor("bo_r", [1, P], F16, kind="ExternalInput")
    ones = nc.dram_tensor("ones", [1, P], F16, kind="ExternalInput")
    outT = nc.dram_tensor("outT", [P, NB * P], F32, kind="ExternalOutput")

    with tile.TileContext(nc) as tc:
        with tile_pools(tc) as (rpool, spool, wpool, apsum, opsum):
            uidx_sb = rpool.tile([P, nsl // 16], I16, tag="uidx")
            nc.sync.dma_start(uidx_sb[:], uidx[:])
            exp_sb = rpool.tile([P, nch * H], F16, tag="exp_sb")
            nc.sync.dma_start(exp_sb[:], exp_in[:])
            wo_sb = rpool.tile([P, P], F16, tag="wo")
            nc.sync.dma_start(wo_sb[:], WoT[:])
            bo_sb = rpool.tile([1, P], F16, tag="bo")
            nc.sync.dma_start(bo_sb[:], bo_r[:])
            ones_sb = rpool.tile([1, P], F16, tag="ones")
            nc.sync.dma_start(ones_sb[:], ones[:])
            aggT = rpool.tile([P, NB * P], F32, tag="aggT")
            nc.vector.memset(aggT[:], 0)

            kg_tiles = {}
            for qi, (c0, nch_c, lohi) in enumerate(cmap.gather_calls()):
                ug = spool.tile([P, GB * P], F16, tag="u_g")
                src_ap = u_full[0:LOHI, :] if lohi == 0 else u_full[LOHI:NPAD, :]
                nc.gpsimd.dma_gather(
                    out_ap=ug[:, :nch_c * P].rearrange("p (s f) -> p s f", f=P),
                    in_ap=src_ap,
                    idxs_ap=uidx_sb[:, c0 * P // 16:(c0 + nch_c) * P // 16],
                    num_idxs=nch_c * P,
                    num_idxs_reg=nch_c * P,
                    elem_size=P,
                    single_packet=False,
                    queue_num=qi % 2,
                )
                kg_tiles[c0] = ug

            tt_tiles = {}

            def stream_tile(tiles, dram, ci):
                b0 = ci // SB * SB
                if b0 not in tiles:
                    t = spool.tile([P, SB * P], F8, tag=dram.name, name=f"strm_{dram.name}_{b0}")
                    n = min(SB, nch - b0) * P
                    nc.sync.dma_start(t[:, :n], dram[:, b0 * P:b0 * P + n])
                    tiles[b0] = t
                return tiles[b0][:, (ci - b0) * P:(ci - b0 + 1) * P]

            for cb0 in range(0, nch, PB):
                cbn = min(PB, nch - cb0)
                erep = wpool.tile([P, PB * P], F16, tag="erep")
                nc.scalar.copy(
                    erep[:, :cbn * P].rearrange("p (c h d) -> p c h d", h=H, d=Dh),
                    exp_sb[:, cb0 * H:(cb0 + cbn) * H]
                    .rearrange("p (c h) -> p c h", h=H)[:, :, :, None]
                    .broadcast_to([P, cbn, H, Dh]))
                wv = wpool.tile([P, PB * P], F16, tag="wv")
                ci = cb0
                while ci < cb0 + cbn:
                    gkey = max(s for s in kg_tiles if s <= ci)
                    cj = min(cb0 + cbn,
                             gkey + min(GB, cmap.region_len(gkey)))
                    n = cj - ci
                    off = (ci - gkey) * P
                    nc.vector.tensor_mul(
                        wv[:, (ci - cb0) * P:(ci - cb0 + n) * P],
                        kg_tiles[gkey][:, off:off + n * P],
                        erep[:, (ci - cb0) * P:(ci - cb0 + n) * P])
                    ci = cj
                ci = cb0
                while ci < cb0 + cbn:
                    hf, blk = cmap.chunks[ci]
                    cj = ci
                    while cj + 1 < cb0 + cbn and cmap.chunks[cj + 1] == (hf, blk):
                        cj += 1
                    agg_ps = apsum.tile([P, P], F32, tag="agg")
                    for ck in range(ci, cj + 1):
                        nc.tensor.matmul(
                            agg_ps[:],
                            lhsT=wv[:, (ck - cb0) * P:(ck - cb0 + 1) * P],
                            rhs=stream_tile(tt_tiles, TT_st, ck),
                            start=(ck == ci), stop=(ck == cj))
                    nc.vector.tensor_add(
                        out=aggT[:, blk * P:(blk + 1) * P],
                        in0=aggT[:, blk * P:(blk + 1) * P],
                        in1=agg_ps[:])
                    ci = cj + 1

            osb = rpool.tile([P, NB * P], F32, tag="osb", bufs=1)
            for blk in range(NB):
                agg16 = wpool.tile([P, P], F16, tag="agg16")
                nc.vector.tensor_copy(agg16[:], aggT[:, blk * P:(blk + 1) * P])
                ops = opsum.tile([P, P], F32, tag="outp")
                nc.tensor.matmul(ops[:], lhsT=wo_sb[:], rhs=agg16[:],
                                 start=True, stop=False)
                nc.tensor.matmul(ops[:], lhsT=bo_sb[:], rhs=ones_sb[:],
                                 start=False, stop=True)
                nc.scalar.copy(osb[:, blk * P:(blk + 1) * P], ops[:])
            nc.sync.dma_start(outT[:], osb[:])
    nc.compile()
    return nc


# ---------------------------------------------------------------- orchestration
def _prep_weights(Wq, bq, Wk, bk, Wv, bv, Wo, bo):
    w16 = {k: np.asarray(v, np.float32).astype(np.float16)
           for k, v in (("Wq", Wq), ("Wk", Wk), ("Wv", Wv), ("Wo", Wo))}
    b16 = {k: np.asarray(v, np.float32).astype(np.float16)
           for k, v in (("bq", bq), ("bk", bk), ("bv", bv), ("bo", bo))}
    return w16, b16


def kernel(node_features, edge_index, Wq, bq, Wk, bk, Wv, bv, Wo, bo):
    node_features = np.asarray(node_features, np.float32)
    edge_index = np.asarray(edge_index)
    src, dst = edge_index[0].astype(np.int64), edge_index[1].astype(np.int64)
    x16 = node_features.astype(np.float16)
    w16, b16 = _prep_weights(Wq, bq, Wk, bk, Wv, bv, Wo, bo)
    ones_row = np.ones((1, P), np.float16)
    cores = list(range(C))

    # ---------------- L1
    nc1 = build_l1()
    in1 = []
    for c in cores:
        base, ln = shard_base(c), shard_len(c)
        xt = np.zeros((P, NB * P), np.float16)
        xt[:, :ln] = x16[base:base + ln].T
        in1.append(dict(
            xT=xt,
            wqkv=np.concatenate([w16["Wq"].T, w16["Wk"].T, w16["Wv"].T],
                                axis=1).copy(),
            bqkv=np.concatenate([b16["bq"], b16["bk"], b16["bv"]])
            .reshape(1, 3 * P), ones=ones_row))
    r1 = run_bass_kernel_spmd(nc1, in1, core_ids=cores)

    k_full = np.zeros((NPAD, P), np.float16)
    for c in cores:
        base, ln = shard_base(c), shard_len(c)
        k_full[base:base + ln] = r1.results[c]["k_sh"][:ln]

    # ---------------- L2
    eids = np.arange(E, dtype=np.int64)
    cmap2 = compute_cmap(src, dst)
    plans2 = []
    for c in cores:
        base, ln = shard_base(c), shard_len(c)
        m = (src >= base) & (src < base + ln)
        plans2.append(CorePlan(cmap2, c, src[m], dst[m], eids[m]))

    nc2 = build_l2(cmap2)
    in2 = []
    for c in cores:
        pl = plans2[c]
        in2.append(dict(
            q_sh=r1.results[c]["q_sh"], v_sh=r1.results[c]["v_sh"],
            k_full=k_full,
            S_st=pl.onehot_stream(False), ST_st=pl.onehot_stream(True),
            kidx=pack_idx16(pl.slot_gidx.astype(np.int16))))
    r2 = run_bass_kernel_spmd(nc2, in2, core_ids=cores)

    exp_edge = np.zeros((E, H), np.float16)
    u_full = np.zeros((NPAD, P), np.float16)
    for c in cores:
        pl = plans2[c]
        exp_flat = r2.results[c]["exp_out"].reshape(P, cmap2.nch, H) \
            .transpose(1, 0, 2).reshape(cmap2.nslots, H)
        real = pl.slot_edge >= 0
        exp_edge[pl.slot_edge[real]] = exp_flat[real]
        base, ln = shard_base(c), shard_len(c)
        u_full[base:base + ln] = r2.results[c]["u_out"][:ln]
    # zero-degree nodes give inf u-rows (1/0); they are never gathered by a
    # real edge, but dummy slots gather row 0 — sanitize so inf*0 can't occur.
    u_full[~np.isfinite(u_full).all(axis=1)] = 0

    # ---------------- L3
    cmap3 = compute_cmap(dst, src)
    plans3 = []
    for c in cores:
        base, ln = shard_base(c), shard_len(c)
        m = (dst >= base) & (dst < base + ln)
        plans3.append(CorePlan(cmap3, c, dst[m], src[m], eids[m]))

    nc3 = build_l3(cmap3)
    in3 = []
    for c in cores:
        pl = plans3[c]
        exp_slots = np.zeros((cmap3.nslots, H), np.float16)
        real = pl.slot_edge >= 0
        exp_slots[real] = exp_edge[pl.slot_edge[real]]
        exp_in = exp_slots.reshape(cmap3.nch, P, H).transpose(1, 0, 2) \
            .reshape(P, cmap3.nch * H)
        in3.append(dict(
            u_full=u_full, TT_st=pl.onehot_stream(True),
            uidx=pack_idx16(pl.slot_gidx.astype(np.int16)),
            exp_in=exp_in, WoT=w16["Wo"].T.copy(),
            bo_r=b16["bo"].reshape(1, P), ones=ones_row))
    r3 = run_bass_kernel_spmd(nc3, in3, core_ids=cores)

    out = np.zeros((N, F), np.float32)
    for c in cores:
        base, ln = shard_base(c), shard_len(c)
        out[base:base + ln] = r3.results[c]["outT"].T[:ln]
    return out



# revision 3
# speedup vs baseline: 1.8411x; 1.8411x over previous
"""Trainium2 Bass kernel for nn_EnhancedReflectiveCognitiveGraph (GNN edge-softmax attention).

Math (see reference):
  q/k/v = x @ W{q,k,v}.T + b ; per-edge scores s_e = <q[src_e], k[dest_e]>_head / 4
  softmax over edges sharing src; agg[dest] += w_e * v[src_e]; out = agg @ Wo.T + bo

Device strategy (8 cores, three SPMD launches, all per-edge data delivered as
sequential HBM streams -- no dma_gather anywhere):
  Host packing: nodes are bin-packed into 392 balanced blocks (<=128 nodes,
      ~equal edge counts) separately for the src phase (L2) and dest phase
      (L3); 49 blocks per core.  Each block owns K 128-slot chunks (uniform K
      across blocks/cores so one program serves all 8 cores).  All per-edge
      operands are host-gathered into slot order and streamed contiguously:
      each SBUF partition's columns are contiguous in DRAM, so every DMA runs
      at full modeled bandwidth.
  L1 (proj): per-core q/k/v for its (src-packed) node shard, one fused
      [128, 3*128] output tile per block, single contiguous store.
  L2 (src phase): per-chunk one-hot matmuls expand q to edges in
      feature-major orientation (qeT[f,e]); k arrives feature-major as a
      host-gathered stream; DVE multiplies; a tiny one-hot head matrix (Hmat)
      reduces heads on the PE (back to slot-major); exp on ACT; segment sums
      per src block via S^T matmuls accumulate in PSUM; reciprocal (+eps via
      an extra accumulate row to avoid inf) folds the softmax denominator
      into per-edge weights w = exp * recip[src], written compactly (E x H).
  L3 (dest phase): v arrives slot-major as a host-gathered stream; w arrives
      compactly and is broadcast along head_dim on ACT; DVE multiplies;
      per-chunk one-hot matmuls scatter-add into per-dest-block agg in a
      single PSUM accumulation run per block; output projection per block.
  Host between launches does pure relayout (gather/permute/pad/zero/concat).
"""

import heapq
import math
import ml_dtypes
import numpy as np

import concourse.bacc as bacc
import concourse.mybir as mybir
import concourse.tile as tile
from concourse.bass_utils import run_bass_kernel_spmd

# ---------------------------------------------------------------- constants
N = 50000
E = 600000
F = 128
H = 8
Dh = 16
P = 128
C = 8                     # cores
NB = 49                   # blocks per core
NBLK = C * NB             # total blocks
SB = 64                   # chunks per stream DMA piece
EPS = 2e-5                # segment-sum epsilon (keeps 1/seg finite in f16)
F16 = mybir.dt.float16
F8 = mybir.dt.float8e4
F32 = mybir.dt.float32


# ---------------------------------------------------------------- host packing
class ChunkMap:
    """Node->(core, block, loc) packing for one phase plus per-core slot plans.

    Blocks are balanced by edge count (LPT) under a 128-node capacity, so the
    uniform per-block chunk count K = ceil(max block edges / 128) is minimal.
    Slot s of chunk c lives at SBUF partition s, stream column c*128+s.
    """

    def __init__(self, key, other):
        key = np.asarray(key, np.int64)
        other = np.asarray(other, np.int64)
        deg = np.bincount(key, minlength=N).astype(np.int64)

        # --- nodes -> 392 blocks (LPT by degree, capacity 128 nodes)
        order = np.argsort(-deg, kind="stable")
        node_gblk = np.empty(N, np.int32)
        node_loc = np.empty(N, np.int32)
        heap = [(0, 0, b) for b in range(NBLK)]
        heapq.heapify(heap)
        for nd in order:
            while True:
                load, cnt, b = heapq.heappop(heap)
                if cnt < P:
                    break
            node_gblk[nd] = b
            node_loc[nd] = cnt
            heapq.heappush(heap, (load + int(deg[nd]), cnt + 1, b))

        blk_load = np.zeros(NBLK, np.int64)
        np.add.at(blk_load, node_gblk[key], 1)

        # --- blocks -> 8 cores (LPT, 49 each)
        blk_core = np.empty(NBLK, np.int32)
        blk_local = np.empty(NBLK, np.int32)
        cheap = [(0, 0, c) for c in range(C)]
        heapq.heapify(cheap)
        for b in np.argsort(-blk_load, kind="stable"):
            while True:
                load, cnt, c = heapq.heappop(cheap)
                if cnt < NB:
                    break
            blk_core[b] = c
            blk_local[b] = cnt
            heapq.heappush(cheap, (load + int(blk_load[b]), cnt + 1, c))

        self.K = max(1, int(np.ceil(blk_load.max() / P)))
        self.nch = NB * self.K
        self.nsl = self.nch * P
        self.node_core = blk_core[node_gblk]
        self.node_blk = blk_local[node_gblk]
        self.node_loc = node_loc

        # --- per-core slot plans
        ecore = self.node_core[key]
        eblk = self.node_blk[key]
        self.plans = []
        for c in range(C):
            m = np.where(ecore == c)[0]
            blk = eblk[m]
            o = np.argsort(blk, kind="stable")
            eids = m[o]
            blk = blk[o]
            starts = np.searchsorted(blk, np.arange(NB))
            pos = np.arange(len(o)) - starts[blk]
            slot = blk * (self.K * P) + pos
            slot_local = np.full(self.nsl, -1, np.int64)
            slot_other = np.zeros(self.nsl, np.int64)
            slot_edge = np.full(self.nsl, -1, np.int64)
            slot_local[slot] = self.node_loc[key[eids]]
            slot_other[slot] = other[eids]
            slot_edge[slot] = eids
            self.plans.append((slot_local, slot_other, slot_edge))

    def onehot_S(self, c):
        """[128, nsl] fp8: S[loc, slot] = 1 (chunk-block diagonal one-hot)."""
        sl, _, _ = self.plans[c]
        out = np.zeros((P, self.nsl), dtype=ml_dtypes.float8_e4m3)
        v = sl >= 0
        out[sl[v], np.arange(self.nsl)[v]] = 1.0
        return out

    def onehot_ST(self, c):
        """[128, nsl] fp8: ST[slot%128, (slot//128)*128 + loc] = 1."""
        sl, _, _ = self.plans[c]
        out = np.zeros((P, self.nsl), dtype=ml_dtypes.float8_e4m3)
        v = np.where(sl >= 0)[0]
        out[v % P, (v // P) * P + sl[v]] = 1.0
        return out


def compute_cmap(key, other):
    return ChunkMap(np.asarray(key), np.asarray(other))


def _subbatches(K, PB):
    s0 = 0
    while s0 < K:
        yield s0, min(PB, K - s0)
        s0 += PB


# ---------------------------------------------------------------- L1: projections
def build_l1():
    nc = bacc.Bacc("TRN2", target_bir_lowering=False, num_devices=C)
    xT = nc.dram_tensor("xT", [P, NB * P], F16, kind="ExternalInput")
    wqkv = nc.dram_tensor("wqkv", [P, 3 * P], F16, kind="ExternalInput")
    bqkv = nc.dram_tensor("bqkv", [1, 3 * P], F16, kind="ExternalInput")
    ones = nc.dram_tensor("ones", [1, P], F16, kind="ExternalInput")
    qkv_sw = nc.dram_tensor("qkv_sw", [P, NB * 3 * P], F16, kind="ExternalOutput")

    with tile.TileContext(nc) as tc:
        with tc.tile_pool(name="const", bufs=1) as cpool, \
             tc.tile_pool(name="psum", bufs=4, space="PSUM") as ppool:
            w_sb = cpool.tile([P, 3 * P], F16, tag="w")
            nc.sync.dma_start(w_sb[:], wqkv[:])
            b_sb = cpool.tile([1, 3 * P], F16, tag="b")
            nc.sync.dma_start(b_sb[:], bqkv[:])
            ones_sb = cpool.tile([1, P], F16, tag="ones")
            nc.sync.dma_start(ones_sb[:], ones[:])
            xt = cpool.tile([P, NB * P], F16, tag="xT")
            nc.sync.dma_start(xt[:], xT[:])
            osb = cpool.tile([P, NB * 3 * P], F16, tag="osb")
            for b in range(NB):
                ps = ppool.tile([P, 3 * P], F32, tag="proj")
                nc.tensor.matmul(ps[:], lhsT=xt[:, b * P:(b + 1) * P],
                                 rhs=w_sb[:], start=True, stop=False)
                nc.tensor.matmul(ps[:], lhsT=ones_sb[:], rhs=b_sb[:],
                                 start=False, stop=True)
                dst = osb[:, b * 3 * P:(b + 1) * 3 * P]
                if b % 2 == 0:
                    nc.scalar.copy(dst, ps[:])
                else:
                    nc.vector.tensor_copy(dst, ps[:])
            nc.sync.dma_start(qkv_sw[:], osb[:])
    nc.compile()
    return nc


# ---------------------------------------------------------------- stream helper
def _stream(nc, spool, tiles, dram, ci, nch, dtype, tag):
    """Piece-cached stream: returns (tile, piece_base_chunk)."""
    b0 = ci // SB * SB
    if b0 not in tiles:
        t = spool.tile([P, SB * P], dtype, tag=tag, name=f"strm_{tag}_{b0}")
        n = min(SB, nch - b0) * P
        nc.sync.dma_start(t[:, :n], dram[:, b0 * P:b0 * P + n])
        tiles[b0] = t
    return tiles[b0], b0


def _stream_h(nc, spool, tiles, dram, ci, nch, tag):
    """Same, for [P, nch*H] compact-w streams."""
    b0 = ci // SB * SB
    if b0 not in tiles:
        t = spool.tile([P, SB * H], F16, tag=tag, name=f"strmh_{tag}_{b0}")
        n = min(SB, nch - b0) * H
        nc.sync.dma_start(t[:, :n], dram[:, b0 * H:b0 * H + n])
        tiles[b0] = t
    return tiles[b0], b0


# ---------------------------------------------------------------- L2: src phase
def build_l2(cmap):
    K, nch, nsl = cmap.K, cmap.nch, cmap.nsl
    PB = (K + 1) // 2
    nc = bacc.Bacc("TRN2", target_bir_lowering=False, num_devices=C)
    q_sw = nc.dram_tensor("q_sw", [P, NB * P], F16, kind="ExternalInput")
    kT_st = nc.dram_tensor("kT_st", [P, nsl], F16, kind="ExternalInput")
    S_st = nc.dram_tensor("S_st", [P, nsl], F8, kind="ExternalInput")
    ST_st = nc.dram_tensor("ST_st", [P, nsl], F8, kind="ExternalInput")
    hmat = nc.dram_tensor("hmat", [P, H], F8, kind="ExternalInput")
    epsc = nc.dram_tensor("epsc", [1, P], F32, kind="ExternalInput")
    ones8 = nc.dram_tensor("ones8", [1, H], F32, kind="ExternalInput")
    w_out = nc.dram_tensor("w_out", [P, nch * H], F16, kind="ExternalOutput")

    with tile.TileContext(nc) as tc:
        with tc.tile_pool(name="res", bufs=1) as rpool, \
             tc.tile_pool(name="stream", bufs=2) as spool, \
             tc.tile_pool(name="work", bufs=2) as wpool, \
             tc.tile_pool(name="small", bufs=2) as mpool, \
             tc.tile_pool(name="qe_ps", bufs=2, space="PSUM") as qpsum, \
             tc.tile_pool(name="sc_ps", bufs=2, space="PSUM") as spsum, \
             tc.tile_pool(name="seg_ps", bufs=1, space="PSUM") as gpsum, \
             tc.tile_pool(name="w_ps", bufs=1, space="PSUM") as wpsum:
            q_sb = rpool.tile([P, NB * P], F16, tag="q_sb")
            nc.sync.dma_start(q_sb[:], q_sw[:])
            hm_sb = rpool.tile([P, H], F8, tag="hm")
            nc.sync.dma_start(hm_sb[:], hmat[:])
            eps_sb = rpool.tile([1, P], F32, tag="eps")
            nc.sync.dma_start(eps_sb[:], epsc[:])
            on8_sb = rpool.tile([1, H], F32, tag="on8")
            nc.sync.dma_start(on8_sb[:], ones8[:])
            exp_sb = rpool.tile([P, nch * H], F16, tag="exp_sb")
            w_sb = rpool.tile([P, nch * H], F16, tag="w_sb")

            kt_tiles, s_tiles, st_tiles = {}, {}, {}

            def S_(ci):
                t, b0 = _stream(nc, spool, s_tiles, S_st, ci, nch, F8, "S")
                return t[:, (ci - b0) * P:(ci - b0 + 1) * P]

            def ST_(ci):
                t, b0 = _stream(nc, spool, st_tiles, ST_st, ci, nch, F8, "ST")
                return t[:, (ci - b0) * P:(ci - b0 + 1) * P]

            for b in range(NB):
                c0 = b * K
                sc = spsum.tile([P, K * H], F32, tag="sc")
                for s0, sn in _subbatches(K, PB):
                    qe = qpsum.tile([P, PB * P], F32, tag="qe")
                    for j in range(sn):
                        nc.tensor.matmul(
                            qe[:, j * P:(j + 1) * P],
                            lhsT=q_sb[:, b * P:(b + 1) * P],
                            rhs=S_(c0 + s0 + j), start=True, stop=True)
                    qk = wpool.tile([P, PB * P], F16, tag="qk")
                    use_act = b % 5 > 0
                    if use_act:
                        qe16 = wpool.tile([P, PB * P], F16, tag="qe16")
                        nc.scalar.copy(qe16[:, :sn * P], qe[:, :sn * P])
                    ci = c0 + s0
                    bnd = c0 + s0 + sn
                    while ci < bnd:
                        t, b0 = _stream(nc, spool, kt_tiles, kT_st, ci, nch,
                                        F16, "kT")
                        cj = min(bnd, b0 + SB)
                        ksl = t[:, (ci - b0) * P:(cj - b0) * P]
                        osl = qk[:, (ci - c0 - s0) * P:(cj - c0 - s0) * P]
                        if use_act:
                            nc.vector.tensor_mul(
                                osl, qe16[:, (ci - c0 - s0) * P:(cj - c0 - s0) * P], ksl)
                        else:
                            nc.vector.scalar_tensor_tensor(
                                out=osl,
                                in0=qe[:, (ci - c0 - s0) * P:(cj - c0 - s0) * P],
                                scalar=1.0, in1=ksl,
                                op0=mybir.AluOpType.mult,
                                op1=mybir.AluOpType.mult)
                        ci = cj
                    for j in range(sn):
                        nc.tensor.matmul(
                            sc[:, (s0 + j) * H:(s0 + j + 1) * H],
                            lhsT=qk[:, j * P:(j + 1) * P],
                            rhs=hm_sb[:], start=True, stop=True)
                nc.scalar.activation(
                    out=exp_sb[:, c0 * H:(c0 + K) * H],
                    in_=sc[:, :K * H],
                    func=mybir.ActivationFunctionType.Exp,
                    scale=1.0 / math.sqrt(Dh))
                seg = gpsum.tile([P, H], F32, tag="seg")
                for j in range(K):
                    nc.tensor.matmul(
                        seg[:], lhsT=ST_(c0 + j),
                        rhs=exp_sb[:, (c0 + j) * H:(c0 + j + 1) * H],
                        start=(j == 0), stop=False)
                nc.tensor.matmul(seg[:], lhsT=eps_sb[:], rhs=on8_sb[:],
                                 start=False, stop=True)
                rec = mpool.tile([P, H], F16, tag="rec")
                with nc.allow_low_precision(reason="1/seg quantized to f16"):
                    nc.vector.reciprocal(rec[:], seg[:])
                wp = wpsum.tile([P, K * H], F32, tag="wp")
                for j in range(K):
                    nc.tensor.matmul(
                        wp[:, j * H:(j + 1) * H],
                        lhsT=S_(c0 + j), rhs=rec[:], start=True, stop=True)
                nc.vector.scalar_tensor_tensor(
                    out=w_sb[:, c0 * H:(c0 + K) * H],
                    in0=exp_sb[:, c0 * H:(c0 + K) * H],
                    scalar=1.0, in1=wp[:, :K * H],
                    op0=mybir.AluOpType.mult, op1=mybir.AluOpType.mult)
            nc.sync.dma_start(w_out[:], w_sb[:])
    nc.compile()
    return nc


# ---------------------------------------------------------------- L3: dest phase
def build_l3(cmap):
    K, nch, nsl = cmap.K, cmap.nch, cmap.nsl
    nc = bacc.Bacc("TRN2", target_bir_lowering=False, num_devices=C)
    v_st = nc.dram_tensor("v_st", [P, nsl], F16, kind="ExternalInput")
    TT_st = nc.dram_tensor("TT_st", [P, nsl], F8, kind="ExternalInput")
    w_cmp = nc.dram_tensor("w_cmp", [P, nch * H], F16, kind="ExternalInput")
    WoT = nc.dram_tensor("WoT", [P, P], F16, kind="ExternalInput")
    bo_r = nc.dram_tensor("bo_r", [1, P], F16, kind="ExternalInput")
    ones = nc.dram_tensor("ones", [1, P], F16, kind="ExternalInput")
    outT = nc.dram_tensor("outT", [P, NB * P], F32, kind="ExternalOutput")

    with tile.TileContext(nc) as tc:
        with tc.tile_pool(name="res", bufs=1) as rpool, \
             tc.tile_pool(name="stream", bufs=2) as spool, \
             tc.tile_pool(name="work", bufs=2) as wpool, \
             tc.tile_pool(name="small", bufs=2) as mpool, \
             tc.tile_pool(name="agg_ps", bufs=2, space="PSUM") as apsum, \
             tc.tile_pool(name="out_ps", bufs=2, space="PSUM") as opsum:
            wc_sb = rpool.tile([P, nch * H], F16, tag="wc")
            nc.sync.dma_start(wc_sb[:], w_cmp[:])
            wo_sb = rpool.tile([P, P], F16, tag="wo")
            nc.sync.dma_start(wo_sb[:], WoT[:])
            bo_sb = rpool.tile([1, P], F16, tag="bo")
            nc.sync.dma_start(bo_sb[:], bo_r[:])
            ones_sb = rpool.tile([1, P], F16, tag="ones")
            nc.sync.dma_start(ones_sb[:], ones[:])
            osb = rpool.tile([P, NB * P], F32, tag="osb")

            v_tiles, tt_tiles = {}, {}

            def TT_(ci):
                t, b0 = _stream(nc, spool, tt_tiles, TT_st, ci, nch, F8, "TT")
                return t[:, (ci - b0) * P:(ci - b0 + 1) * P]

            for b in range(NB):
                c0 = b * K
                w16 = wpool.tile([P, K * P], F16, tag="w16")
                nc.scalar.copy(
                    w16[:].rearrange("p (c h d) -> p c h d", h=H, d=Dh),
                    wc_sb[:, c0 * H:(c0 + K) * H]
                    .rearrange("p (c h) -> p c h", h=H)[:, :, :, None]
                    .broadcast_to([P, K, H, Dh]))
                wv = wpool.tile([P, K * P], F16, tag="wv")
                ci = c0
                while ci < c0 + K:
                    t, b0 = _stream(nc, spool, v_tiles, v_st, ci, nch, F16, "v")
                    cj = min(c0 + K, b0 + SB)
                    nc.vector.tensor_mul(
                        wv[:, (ci - c0) * P:(cj - c0) * P],
                        t[:, (ci - b0) * P:(cj - b0) * P],
                        w16[:, (ci - c0) * P:(cj - c0) * P])
                    ci = cj
                agg = apsum.tile([P, P], F32, tag="agg")
                for j in range(K):
                    nc.tensor.matmul(
                        agg[:], lhsT=wv[:, j * P:(j + 1) * P],
                        rhs=TT_(c0 + j), start=(j == 0), stop=(j == K - 1))
                agg16 = mpool.tile([P, P], F16, tag="agg16")
                nc.vector.tensor_copy(agg16[:], agg[:])
                ops = opsum.tile([P, P], F32, tag="ops")
                nc.tensor.matmul(ops[:], lhsT=wo_sb[:], rhs=agg16[:],
                                 start=True, stop=False)
                nc.tensor.matmul(ops[:], lhsT=bo_sb[:], rhs=ones_sb[:],
                                 start=False, stop=True)
                nc.vector.tensor_copy(osb[:, b * P:(b + 1) * P], ops[:])
            nc.sync.dma_start(outT[:], osb[:])
    nc.compile()
    return nc


# ---------------------------------------------------------------- orchestration
def kernel(node_features, edge_index, Wq, bq, Wk, bk, Wv, bv, Wo, bo):
    node_features = np.asarray(node_features, np.float32)
    edge_index = np.asarray(edge_index)
    src, dst = edge_index[0].astype(np.int64), edge_index[1].astype(np.int64)
    x16 = node_features.astype(np.float16)
    w16 = {k: np.asarray(v, np.float32).astype(np.float16)
           for k, v in (("Wq", Wq), ("Wk", Wk), ("Wv", Wv), ("Wo", Wo))}
    b16 = {k: np.asarray(v, np.float32).astype(np.float16)
           for k, v in (("bq", bq), ("bk", bk), ("bv", bv), ("bo", bo))}
    ones_row = np.ones((1, P), np.float16)
    cores = list(range(C))

    cmap2 = compute_cmap(src, dst)
    cmap3 = compute_cmap(dst, src)

    # ---------------- L1 (node shards use the src-phase packing)
    nc1 = build_l1()
    in1 = []
    core_nodes = [np.where(cmap2.node_core == c)[0] for c in cores]
    for c in cores:
        nd = core_nodes[c]
        xt = np.zeros((P, NB * P), np.float16)
        xt[:, cmap2.node_blk[nd] * P + cmap2.node_loc[nd]] = x16[nd].T
        in1.append(dict(
            xT=xt,
            wqkv=np.concatenate([w16["Wq"].T, w16["Wk"].T, w16["Wv"].T],
                                axis=1).copy(),
            bqkv=np.concatenate([b16["bq"], b16["bk"], b16["bv"]])
            .reshape(1, 3 * P), ones=ones_row))
    r1 = run_bass_kernel_spmd(nc1, in1, core_ids=cores)

    k16 = np.zeros((N, F), np.float16)
    v16 = np.zeros((N, F), np.float16)
    q_sw = []
    for c in cores:
        arr = r1.results[c]["qkv_sw"].reshape(P, NB, 3, P)
        q_sw.append(np.ascontiguousarray(arr[:, :, 0, :].reshape(P, NB * P)))
        nd = core_nodes[c]
        k16[nd] = arr[cmap2.node_loc[nd], cmap2.node_blk[nd], 1, :]
        v16[nd] = arr[cmap2.node_loc[nd], cmap2.node_blk[nd], 2, :]

    # ---------------- L2
    hmat = np.zeros((P, H), dtype=ml_dtypes.float8_e4m3)
    hmat[np.arange(P), np.arange(P) // Dh] = 1.0
    nc2 = build_l2(cmap2)
    in2 = []
    for c in cores:
        sl, so, se = cmap2.plans[c]
        ke = k16[so]                      # [nsl, F]; dummy slots read row 0
        ke[se < 0] = 0
        in2.append(dict(
            q_sw=q_sw[c],
            kT_st=np.ascontiguousarray(ke.T),
            S_st=cmap2.onehot_S(c), ST_st=cmap2.onehot_ST(c),
            hmat=hmat,
            epsc=np.full((1, P), EPS, np.float32),
            ones8=np.ones((1, H), np.float32)))
    r2 = run_bass_kernel_spmd(nc2, in2, core_ids=cores)

    w_edge = np.zeros((E, H), np.float16)
    for c in cores:
        sl, so, se = cmap2.plans[c]
        arr = r2.results[c]["w_out"].reshape(P, cmap2.nch, H) \
            .transpose(1, 0, 2).reshape(cmap2.nsl, H)
        valid = se >= 0
        w_edge[se[valid]] = arr[valid]

    # ---------------- L3
    nc3 = build_l3(cmap3)
    in3 = []
    for c in cores:
        sl, so, se = cmap3.plans[c]
        ve = v16[so]
        ve[se < 0] = 0
        v_strm = ve.reshape(cmap3.nch, P, F).transpose(1, 0, 2) \
            .reshape(P, cmap3.nsl)
        we = np.zeros((cmap3.nsl, H), np.float16)
        valid = se >= 0
        we[valid] = w_edge[se[valid]]
        w_cmp = we.reshape(cmap3.nch, P, H).transpose(1, 0, 2) \
            .reshape(P, cmap3.nch * H)
        in3.append(dict(
            v_st=np.ascontiguousarray(v_strm),
            TT_st=cmap3.onehot_ST(c),
            w_cmp=np.ascontiguousarray(w_cmp),
            WoT=w16["Wo"].T.copy(),
            bo_r=b16["bo"].reshape(1, P), ones=ones_row))
    r3 = run_bass_kernel_spmd(nc3, in3, core_ids=cores)

    out = np.zeros((N, F), np.float32)
    for c in cores:
        nd = np.where(cmap3.node_core == c)[0]
        outT = r3.results[c]["outT"]
        out[nd] = outT[:, cmap3.node_blk[nd] * P + cmap3.node_loc[nd]].T
    return out
